# revision 17
# baseline (speedup 1.0000x reference)
"""MoE router kernel for Trainium2, 8-core data-parallel (token-sharded).

Computes, for x[16384,4096], W[64,4096], b[64]:
  router_logits = x @ W.T + b                      [N, 64] fp32
  router_probs  = softmax(logits)                  (internal)
  top_k_probs, top_k_indices = top_k(probs, 8)     [N, 8]
  top_k_weights = top_k_probs / (sum + 1e-6)       [N, 8] fp32
  expert_mask   = sum_k one_hot(indices)           [N, 64] fp32

Scheme (per core, 2048 tokens):
  - DMA x tiles [128n, 4096h] natural layout.
  - PE transposes 128x128 blocks to PSUM (fp32, exact).
  - The mandatory PSUM->SBUF copies double as an fp16 hi/lo split:
    ACT cast-copy makes xh=fp16(x^T), DVE subtract makes xl=fp16(x^T-xh).
  - W is transposed on-device, scaled by 2^6 (keeps fp16 residual normal),
    split into Wh/Wl fp16 and stacked as one [128h, 128] stationary [Wh|Wl].
    Two fp16 matmuls per h-block (rhs=xh, rhs=xl) accumulate all four
    products (Wh+Wl)(xh+xl) = 64*(x@W.T) in PSUM rows [0:64]+[64:128],
    giving ~2^-22 relative accuracy; the halves are summed after the
    logits transpose, where they land in the free dimension.
  - Logits descaled by 2^-6 with bias added in the PSUM->SBUF copy (exact),
    PE-transposed back to [token, expert] layout.
  - Softmax denominator via ACT exp + DVE reduce; top-8 via DVE max /
    max_index; expert mask via match_replace + compare.
"""
import numpy as np

NUM_TOKENS = 16384
HIDDEN = 4096
NUM_EXPERTS = 64
TOP_K = 8
EPS = 1e-6
WSCALE = 64.0
N_CORES = 8

T = NUM_TOKENS // N_CORES   # 2048 tokens per core
NT = T // 512               # 4 n-tiles
HB = HIDDEN // 128          # 32 h-blocks

_CACHE = {}


def _build(repeat=None):
    """Build the per-core Bass module.  repeat=None: the real kernel.
    repeat=R: wraps the main token loop in a hardware For_i loop executing
    R times (used only for steady-state timing measurements)."""
    import contextlib
    import concourse.bacc as bacc
    import concourse.mybir as mybir
    from concourse.tile import TileContext
    from concourse.masks import make_identity

    f32 = mybir.dt.float32
    f16 = mybir.dt.float16
    u32 = mybir.dt.uint32
    i32 = mybir.dt.int32
    AF = mybir.ActivationFunctionType
    E, K, H = NUM_EXPERTS, TOP_K, HIDDEN

    nc = bacc.Bacc()
    x = nc.dram_tensor("x", [T, H], f32, kind="ExternalInput")
    w = nc.dram_tensor("w", [E, H], f32, kind="ExternalInput")
    b = nc.dram_tensor("b", [1, E], f32, kind="ExternalInput")
    logits_o = nc.dram_tensor("logits", [T, E], f32, kind="ExternalOutput")
    weights_o = nc.dram_tensor("weights", [T, K], f32, kind="ExternalOutput")
    idx_o = nc.dram_tensor("idx", [T, K], i32, kind="ExternalOutput")
    mask_o = nc.dram_tensor("mask", [T, E], f32, kind="ExternalOutput")

    with TileContext(nc) as tc, \
         tc.tile_pool(name="const", bufs=1) as cpool, \
         tc.tile_pool(name="wpool", bufs=1) as wpool, \
         tc.tile_pool(name="xload", bufs=2) as xpool, \
         tc.tile_pool(name="work", bufs=3) as work, \
         tc.tile_pool(name="outp", bufs=2) as outp, \
         tc.tile_pool(name="ps_t", bufs=3, space="PSUM") as ps_t, \
         tc.tile_pool(name="ps_acc", bufs=2, space="PSUM") as ps_acc, \
         tc.tile_pool(name="ps_lt", bufs=2, space="PSUM") as ps_lt:

        ident = cpool.tile([128, 128], f32)
        make_identity(nc, ident[:])

        # ---- W prep: transpose to [H,E], scale by 64, split into fp16 hi/lo
        w_nat = cpool.tile([E, H], f32)
        nc.sync.dma_start(w_nat[:], w[:, :])
        # b2 = [b; 0]: per-partition bias for the stacked [Wh|Wl] PSUM layout
        b_col = cpool.tile([2 * E, 1], f32)
        nc.vector.memset(b_col[:], 0.0)
        nc.sync.dma_start(b_col[0:E, :], b.ap().rearrange("o e -> e o"))

        # whl[:, hb, 0:E] = Wh (fp16 hi of 64*W^T), whl[:, hb, E:2E] = Wl (residual)
        whl = wpool.tile([128, HB, 2 * E], f16)
        for hb in range(HB):
            wt_ps = ps_t.tile([128, E], f32, tag="xtp")
            nc.tensor.transpose(wt_ps[:], w_nat[:, hb * 128:(hb + 1) * 128],
                                ident[:E, :E])
            wt64 = work.tile([128, E], f32, tag="wt64")
            nc.scalar.activation(wt64[:], wt_ps[:], AF.Copy, scale=WSCALE)
            nc.scalar.copy(whl[:, hb, 0:E], wt64[:])
            nc.vector.tensor_sub(whl[:, hb, E:2 * E], wt64[:], whl[:, hb, 0:E])

        # ---- main loop over 512-token tiles
        loop_cm = tc.For_i(0, repeat, 1) if repeat is not None else \
            contextlib.nullcontext()
        with loop_cm:
            _emit_main(nc, tc, mybir, x, logits_o, weights_o, idx_o, mask_o,
                       xpool, work, outp, ps_t, ps_acc, ps_lt,
                       ident, whl, b_col)

    nc.compile()
    return nc


def _emit_main(nc, tc, mybir, x, logits_o, weights_o, idx_o, mask_o,
               xpool, work, outp, ps_t, ps_acc, ps_lt, ident, whl, b_col):
    f32 = mybir.dt.float32
    f16 = mybir.dt.float16
    u32 = mybir.dt.uint32
    i32 = mybir.dt.int32
    AF = mybir.ActivationFunctionType
    E, K, H = NUM_EXPERTS, TOP_K, HIDDEN
    def emit_loads(nt):
        xns = []
        if True:
            for bb in range(4):
                xn = xpool.tile([128, H], f32, tag=f"xn{bb}")
                xr = x.ap().rearrange("(n p) h -> n p h", p=128)[nt * 4 + bb]
                # chunked loads so the first transposes start after ~512KB,
                # not after the full 2MB tile
                for cc in range(4):
                    nc.sync.dma_start(xn[:, cc * (H // 4):(cc + 1) * (H // 4)],
                                      xr[:, cc * (H // 4):(cc + 1) * (H // 4)])
                xns.append(xn)
        return xns

    def emit_mm_loop(nt, xns):
        if True:
            lacc = ps_acc.tile([128, 512], f32, tag="lacc")

            def emit_tr(hb):
                xtp = ps_t.tile([128, 512], f32, tag="xtp")
                for bb in range(4):
                    nc.tensor.transpose(
                        xtp[:, bb * 128:(bb + 1) * 128],
                        xns[bb][:, hb * 128:(hb + 1) * 128], ident[:])
                return xtp

            # software-pipelined: transposes for hb+1 are emitted before the
            # matmuls of hb so the (in-order) PE never idles on the copies
            tr_q = [emit_tr(0)]
            for hb in range(HB):
                if hb + 1 < HB:
                    tr_q.append(emit_tr(hb + 1))
                xtp_cur = tr_q.pop(0)
                xh = work.tile([128, 512], f16, tag="xh")
                nc.scalar.copy(xh[:], xtp_cur[:])
                xl = work.tile([128, 512], f16, tag="xl")
                nc.vector.tensor_sub(xl[:], xtp_cur[:], xh[:])
                # stacked stationary [Wh | Wl]: one matmul per rhs computes both
                # halves; rows 0:E accumulate Wh*rhs, rows E:2E accumulate Wl*rhs
                nc.tensor.matmul(lacc[:], whl[:, hb], xh[:],
                                 start=(hb == 0), stop=False)
                nc.tensor.matmul(lacc[:], whl[:, hb], xl[:],
                                 start=False, stop=(hb == HB - 1))
        return lacc

    def emit_tail(nt, lacc):
        if True:
            # descale + bias on the stacked halves -> SBUF [2E, 512]; the
            # Wh/Wl halves are combined after the transpose (free-dim add)
            lsb = work.tile([2 * E, 512], f32, tag="lsb")
            nc.scalar.activation(lsb[:], lacc[:], AF.Identity,
                                 bias=b_col[:], scale=1.0 / WSCALE)
            L = outp.tile([128, 4, E], f32, tag="L")
            for bb in range(4):
                lt_ps = ps_lt.tile([128, 2 * E], f32, tag="ltps")
                nc.tensor.transpose(lt_ps[:], lsb[:, bb * 128:(bb + 1) * 128],
                                    ident[:])
                lt_lo = work.tile([128, E], f32, tag="lt_lo")
                nc.vector.tensor_copy(lt_lo[:], lt_ps[:, E:2 * E])
                nc.vector.tensor_add(L[:, bb], lt_ps[:, 0:E], lt_lo[:])
            nc.sync.dma_start(
                logits_o.ap().rearrange("(nb p) e -> p nb e", p=128)[
                    :, nt * 4:(nt + 1) * 4], L[:])

            # ---- softmax denominator + top-8 + mask
            Ex = outp.tile([128, 4, E], f32, tag="Ex")
            Z = outp.tile([128, 4], f32, tag="Z")
            nc.scalar.activation(Ex[:], L[:], AF.Exp)
            nc.vector.reduce_sum(Z[:], Ex[:], axis=mybir.AxisListType.X)
            V8 = outp.tile([128, 4, K], f32, tag="V8")
            I8 = outp.tile([128, 4, K], u32, tag="I8")
            MR = outp.tile([128, 4, E], f32, tag="MR")
            for bb in range(4):
                nc.vector.max(out=V8[:, bb], in_=Ex[:, bb])
                nc.vector.max_index(out=I8[:, bb], in_max=V8[:, bb],
                                    in_values=Ex[:, bb])
                nc.vector.match_replace(out=MR[:, bb], in_to_replace=V8[:, bb],
                                        in_values=Ex[:, bb], imm_value=-1.0)
            mask = outp.tile([128, 4, E], f32, tag="mask")
            nc.vector.tensor_scalar(mask[:], MR[:], 0.0, scalar2=None,
                                    op0=mybir.AluOpType.is_lt)
            nc.sync.dma_start(
                mask_o.ap().rearrange("(nb p) e -> p nb e", p=128)[
                    :, nt * 4:(nt + 1) * 4], mask[:])
            S8 = outp.tile([128, 4], f32, tag="S8")
            nc.vector.reduce_sum(S8[:], V8[:], axis=mybir.AxisListType.X)
            den = outp.tile([128, 4], f32, tag="den")
            nc.vector.scalar_tensor_tensor(
                out=den[:], in0=Z[:], scalar=EPS, in1=S8[:],
                op0=mybir.AluOpType.mult, op1=mybir.AluOpType.add)
            rec = outp.tile([128, 4], f32, tag="rec")
            nc.vector.reciprocal(rec[:], den[:])
            W8 = outp.tile([128, 4, K], f32, tag="W8")
            nc.vector.tensor_mul(W8[:], V8[:], rec[:].to_broadcast([128, 4, K]))
            nc.sync.dma_start(
                weights_o.ap().rearrange("(nb p) k -> p nb k", p=128)[
                    :, nt * 4:(nt + 1) * 4], W8[:])
            nc.sync.dma_start(
                idx_o.ap().rearrange("(nb p) k -> p nb k", p=128)[
                    :, nt * 4:(nt + 1) * 4], I8[:].bitcast(i32))

    # driver: tails are deferred one tile so the PE rolls straight from one
    # tile's matmuls into the next tile's transposes without waiting on the
    # ACT logits copy
    prev = None
    for nt in range(NT):
        xns = emit_loads(nt)
        lacc = emit_mm_loop(nt, xns)
        if prev is not None:
            emit_tail(prev[0], prev[1])
        prev = (nt, lacc)
    emit_tail(prev[0], prev[1])


def kernel(x, W, b):
    x = np.ascontiguousarray(np.asarray(x), dtype=np.float32)
    W = np.ascontiguousarray(np.asarray(W), dtype=np.float32)
    b = np.ascontiguousarray(np.asarray(b), dtype=np.float32).reshape(1, NUM_EXPERTS)

    if "nc" not in _CACHE:
        _CACHE["nc"] = _build()
    nc = _CACHE["nc"]

    from concourse.bass_utils import run_bass_kernel_spmd
    in_maps = [
        {"x": x[c * T:(c + 1) * T], "w": W, "b": b}
        for c in range(N_CORES)
    ]
    res = run_bass_kernel_spmd(nc, in_maps, core_ids=list(range(N_CORES)))
    logits = np.concatenate([r["logits"] for r in res.results], axis=0)
    weights = np.concatenate([r["weights"] for r in res.results], axis=0)
    idx = np.concatenate([r["idx"] for r in res.results], axis=0).astype(np.int32)
    mask = np.concatenate([r["mask"] for r in res.results], axis=0)
    return logits, weights, idx, mask


if __name__ == "__main__":
    rng = np.random.default_rng(0)
    x = rng.standard_normal((NUM_TOKENS, HIDDEN), dtype=np.float32)
    W = (rng.standard_normal((NUM_EXPERTS, HIDDEN), dtype=np.float32)
         * HIDDEN ** -0.5).astype(np.float32)
    b = (rng.standard_normal((NUM_EXPERTS,), dtype=np.float32) * 0.01)
    outs = kernel(x=x, W=W, b=b)
    for o in outs:
        print(o.shape, o.dtype)

